# revision 20
# baseline (speedup 1.0000x reference)
"""NoisyTopkRouter Trainium2 kernel.

Computes, for x [8, 4096, 1024] f32, W [64, 1024], b [64]:
  logits = x @ W.T + b            [8, 4096, 64]
  top_vals, indices = top_k(logits, 8)
  probs = softmax(scatter(-inf, indices, top_vals))   (zeros off the top-8)
Returns (probs [8,4096,64] f32, indices [8,4096,8] int32).

Sharding: data-parallel over the batch dim — core i handles x[i] (4096
tokens).  Router weights are replicated.

All DRAM tensors are staged host-side as exact SBUF images so tiles move
with single large-descriptor DMAs:
  xs [NST*128, DC*ST]  xs[s*128+p, c*ST+t] = x[s*ST+t, 128c+p]
  ws [128, DC*E]       ws[p, c*E+e]        = W[e, 128c+p]
  bt [128, SUB*E]      bias replicated across partitions and subtiles
  probs_raw [NST*128, SUB*E],  idx_raw [NST*128, SUB*K]  (unshuffled on host)

The softmax skips the usual max-subtraction: logits here are bounded
(|logit| <= 32 * max|x| * max|w| ~ 5), so exp() cannot overflow and
exp(v)/sum(exp(top8)) is algebraically identical to the max-shifted form.
"""

import sys

sys.path.insert(0, "/opt/trn_rl_repo")

import numpy as np

D = 1024
E = 64
K = 8
P = 128
DC = D // P  # 8 d-chunks of 128
N_CORES = 8
ST = 512  # tokens per supertile (one DMA batch)
SUB = ST // P  # 128-token subtiles per supertile


def build_nc(T=4096):
    """Build + compile the per-core program for T tokens."""
    import concourse.bass as bass  # noqa: F401
    import concourse.tile as tile
    from concourse import bacc, mybir

    f32 = mybir.dt.float32
    u32 = mybir.dt.uint32
    Alu = mybir.AluOpType
    Act = mybir.ActivationFunctionType
    Axis = mybir.AxisListType

    assert T % ST == 0
    NST = T // ST

    nc = bacc.Bacc("TRN2", target_bir_lowering=False, debug=False)

    xs = nc.dram_tensor("xs", [NST * P, DC * ST], f32, kind="ExternalInput").ap()
    ws = nc.dram_tensor("ws", [P, DC * E], f32, kind="ExternalInput").ap()
    bt = nc.dram_tensor("bt", [P, SUB * E], f32, kind="ExternalInput").ap()
    praw = nc.dram_tensor("praw", [NST * P, SUB * E], f32, kind="ExternalOutput").ap()
    iraw = nc.dram_tensor("iraw", [NST * P, SUB * K], u32, kind="ExternalOutput").ap()

    from contextlib import ExitStack

    with tile.TileContext(nc) as tc, ExitStack() as ctx:
        const = ctx.enter_context(tc.tile_pool(name="const", bufs=1))
        xpool = ctx.enter_context(tc.tile_pool(name="x", bufs=3))
        ppool = ctx.enter_context(tc.tile_pool(name="ps", bufs=4, space="PSUM"))
        opool = ctx.enter_context(tc.tile_pool(name="out", bufs=3))
        work = ctx.enter_context(tc.tile_pool(name="work", bufs=3))
        small = ctx.enter_context(tc.tile_pool(name="small", bufs=4))

        ws_sb = const.tile([P, DC * E], f32)
        nc.sync.dma_start(ws_sb[:], ws[:])
        bt_sb = const.tile([P, SUB * E], f32)
        nc.sync.dma_start(bt_sb[:], bt[:])

        for s in range(NST):
            # supertile in 4 token-quarter DMAs so subtile u's matmuls can
            # start as soon as its 512KB lands (512B runs: full DMA rate)
            xt = xpool.tile([P, DC * ST], f32, tag="xst")
            xt3 = xt[:].rearrange("p (c t) -> p c t", t=ST)
            xs3 = xs[s * P : (s + 1) * P, :].rearrange("p (c t) -> p c t", t=ST)
            for u in range(SUB):
                nc.sync.dma_start(
                    xt3[:, :, u * P : (u + 1) * P], xs3[:, :, u * P : (u + 1) * P]
                )
            po = opool.tile([P, SUB * E], f32, tag="po")
            io = opool.tile([P, SUB * K], u32, tag="io")
            # all 4 subtiles' logits into one PSUM bank [128, 256]
            ps4 = ppool.tile([P, SUB * E], f32, tag="ps")
            for u in range(SUB):
                for c in range(DC):
                    nc.tensor.matmul(
                        ps4[:, u * E : (u + 1) * E],
                        lhsT=xt[:, c * ST + u * P : c * ST + (u + 1) * P],
                        rhs=ws_sb[:, c * E : (c + 1) * E],
                        start=(c == 0),
                        stop=(c == DC - 1),
                    )
            # tail over subtile range [u0, u0+nsub): batched for steady-state
            # supertiles, per-subtile for the last one so the pipeline drains
            def tail(u0, nsub):
                lgn = work.tile([P, SUB * E], f32, tag="lg")
                lg = lgn[:, : nsub * E]
                nc.vector.tensor_add(
                    lg, ps4[:, u0 * E : (u0 + nsub) * E], bt_sb[:, : nsub * E]
                )
                v8n = small.tile([P, SUB * K], f32, tag="v8")
                for j in range(nsub):
                    nc.vector.max(
                        out=v8n[:, j * K : (j + 1) * K],
                        in_=lg[:, j * E : (j + 1) * E],
                    )
                # shared per-row shift keeps every exp argument in [-x, 0];
                # any per-row constant cancels in the softmax ratio
                nm = small.tile([P, 1], f32, tag="nm")
                nc.vector.tensor_reduce(
                    nm[:], lg, axis=Axis.X, op=Alu.max, negate=True
                )
                for j in range(nsub):
                    nc.vector.max_index(
                        io[:, (u0 + j) * K : (u0 + j + 1) * K],
                        v8n[:, j * K : (j + 1) * K],
                        lg[:, j * E : (j + 1) * E],
                    )
                # denominators: sum over each subtile's top-8 exps
                e8n = small.tile([P, SUB * K], f32, tag="e8")
                nc.scalar.activation(
                    e8n[:, : nsub * K], v8n[:, : nsub * K], Act.Exp, bias=nm[:]
                )
                s8n = small.tile([P, SUB], f32, tag="s8")
                nc.vector.tensor_reduce(
                    s8n[:, :nsub],
                    e8n[:, : nsub * K].rearrange("p (u k) -> p u k", k=K),
                    axis=Axis.X,
                    op=Alu.add,
                )
                rcn = small.tile([P, SUB], f32, tag="rc")
                nc.vector.reciprocal(rcn[:, :nsub], s8n[:, :nsub])
                exn = work.tile([P, SUB * E], f32, tag="ex")
                nc.scalar.activation(exn[:, : nsub * E], lg, Act.Exp, bias=nm[:])
                mkn = work.tile([P, SUB * E], f32, tag="mk")
                for j in range(nsub):
                    # (logit >= 8th-max) * 1/sum8  in one op
                    nc.vector.tensor_scalar(
                        out=mkn[:, j * E : (j + 1) * E],
                        in0=lg[:, j * E : (j + 1) * E],
                        scalar1=v8n[:, j * K + K - 1 : j * K + K],
                        scalar2=rcn[:, j : j + 1],
                        op0=Alu.is_ge,
                        op1=Alu.mult,
                    )
                nc.vector.tensor_mul(
                    po[:, u0 * E : (u0 + nsub) * E],
                    exn[:, : nsub * E],
                    mkn[:, : nsub * E],
                )

            tail(0, SUB)
            # SWDGE (gpsimd) queues: keeps stores off the SP sequencer so the
            # next supertile's x load is never head-of-line blocked.
            nc.gpsimd.dma_start(praw[s * P : (s + 1) * P, :], po[:])
            nc.gpsimd.dma_start(iraw[s * P : (s + 1) * P, :], io[:])

    nc.compile()
    return nc


_NC_CACHE = {}


def _get_nc(T=4096):
    if T not in _NC_CACHE:
        _NC_CACHE[T] = build_nc(T)
    return _NC_CACHE[T]


def _stage_inputs(x, W, b):
    x = np.asarray(x, dtype=np.float32)
    W = np.asarray(W, dtype=np.float32)
    b = np.asarray(b, dtype=np.float32)
    B, S, d = x.shape
    assert d == D and W.shape == (E, D) and b.shape == (E,)
    NST = S // ST
    # ws[p, c*E+e] = W[e, 128c+p]
    ws = np.ascontiguousarray(W.T.reshape(DC, P, E).transpose(1, 0, 2).reshape(P, DC * E))
    bt = np.ascontiguousarray(np.broadcast_to(np.tile(b, SUB)[None, :], (P, SUB * E)))
    in_maps = []
    for i in range(N_CORES):
        # xs[s*128+p, c*ST+t] = x[i, s*ST+t, 128c+p]
        xi = x[i].reshape(NST, ST, DC, P)  # [s, t, c, p]
        xsi = np.ascontiguousarray(xi.transpose(0, 3, 2, 1).reshape(NST * P, DC * ST))
        in_maps.append({"xs": xsi, "ws": ws, "bt": bt})
    return in_maps


def _unstage_outputs(res, S):
    NST = S // ST
    probs = np.empty((N_CORES, S, E), dtype=np.float32)
    indices = np.empty((N_CORES, S, K), dtype=np.int32)
    for i in range(N_CORES):
        pr = res[i]["praw"].reshape(NST, P, SUB, E).transpose(0, 2, 1, 3)
        probs[i] = pr.reshape(S, E)
        ir = res[i]["iraw"].view(np.int32).reshape(NST, P, SUB, K).transpose(0, 2, 1, 3)
        indices[i] = ir.reshape(S, K)
    return probs, indices


def kernel(x, W, b):
    from concourse.bass_utils import run_bass_kernel_spmd

    x = np.asarray(x, dtype=np.float32)
    B, S, d = x.shape
    assert B == N_CORES
    nc = _get_nc(S)
    in_maps = _stage_inputs(x, W, b)
    res = run_bass_kernel_spmd(nc, in_maps, list(range(N_CORES))).results
    return _unstage_outputs(res, S)


# revision 28
# speedup vs baseline: 1.0042x; 1.0042x over previous
"""NoisyTopkRouter Trainium2 kernel.

Computes, for x [8, 4096, 1024] f32, W [64, 1024], b [64]:
  logits = x @ W.T + b            [8, 4096, 64]
  top_vals, indices = top_k(logits, 8)
  probs = softmax(scatter(-inf, indices, top_vals))   (zeros off the top-8)
Returns (probs [8,4096,64] f32, indices [8,4096,8] int32).

Sharding: data-parallel over the batch dim — core i handles x[i] (4096
tokens).  Router weights are replicated.

All DRAM tensors are staged host-side as exact SBUF images so tiles move
with single large-descriptor DMAs:
  xs [NST*128, DC*ST]  xs[s*128+p, c*ST+t] = x[s*ST+t, 128c+p]
  ws [128, DC*E]       ws[p, c*E+e]        = W[e, 128c+p]
  bt [128, SUB*E]      bias replicated across partitions and subtiles
  probs_raw [NST*128, SUB*E],  idx_raw [NST*128, SUB*K]  (unshuffled on host)

The softmax subtracts one shared per-row max (over the whole batched
tile) instead of a per-subtile max: any per-row constant cancels in the
exp(v)/sum(exp(top8)) ratio, and a single shift keeps the cross-engine
dependency chain short.
"""

import sys

try:  # the axon boot usually provides concourse already
    import concourse  # noqa: F401
except ImportError:
    sys.path.insert(0, "/opt/trn_rl_repo")

import numpy as np

D = 1024
E = 64
K = 8
P = 128
DC = D // P  # 8 d-chunks of 128
N_CORES = 8
ST = 512  # tokens per supertile (one DMA batch)
SUB = ST // P  # 128-token subtiles per supertile


def build_nc(T=4096):
    """Build + compile the per-core program for T tokens."""
    import concourse.bass as bass  # noqa: F401
    import concourse.tile as tile
    from concourse import bacc, mybir

    f32 = mybir.dt.float32
    u32 = mybir.dt.uint32
    Alu = mybir.AluOpType
    Act = mybir.ActivationFunctionType
    Axis = mybir.AxisListType

    assert T % ST == 0
    NST = T // ST

    nc = bacc.Bacc("TRN2", target_bir_lowering=False, debug=False)

    xs = nc.dram_tensor("xs", [NST * P, DC * ST], f32, kind="ExternalInput").ap()
    ws = nc.dram_tensor("ws", [P, DC * E], f32, kind="ExternalInput").ap()
    bt = nc.dram_tensor("bt", [P, SUB * E], f32, kind="ExternalInput").ap()
    praw = nc.dram_tensor("praw", [NST * P, SUB * E], f32, kind="ExternalOutput").ap()
    iraw = nc.dram_tensor("iraw", [NST * P, SUB * K], u32, kind="ExternalOutput").ap()

    from contextlib import ExitStack

    with tile.TileContext(nc) as tc, ExitStack() as ctx:
        const = ctx.enter_context(tc.tile_pool(name="const", bufs=1))
        xpool = ctx.enter_context(tc.tile_pool(name="x", bufs=4))
        ppool = ctx.enter_context(tc.tile_pool(name="ps", bufs=8, space="PSUM"))
        opool = ctx.enter_context(tc.tile_pool(name="out", bufs=3))
        work = ctx.enter_context(tc.tile_pool(name="work", bufs=3))
        small = ctx.enter_context(tc.tile_pool(name="small", bufs=4))

        ws_sb = const.tile([P, DC * E], f32)
        nc.sync.dma_start(ws_sb[:], ws[:])
        bt_sb = const.tile([P, SUB * E], f32)
        nc.sync.dma_start(bt_sb[:], bt[:])

        for s in range(NST):
            # supertile in 4 token-quarter DMAs so subtile u's matmuls can
            # start as soon as its 512KB lands (512B runs: full DMA rate)
            xt = xpool.tile([P, DC * ST], f32, tag="xst")
            xt3 = xt[:].rearrange("p (c t) -> p c t", t=ST)
            xs3 = xs[s * P : (s + 1) * P, :].rearrange("p (c t) -> p c t", t=ST)
            for u in range(SUB):
                nc.sync.dma_start(
                    xt3[:, :, u * P : (u + 1) * P], xs3[:, :, u * P : (u + 1) * P]
                )
            po = opool.tile([P, SUB * E], f32, tag="po")
            io = opool.tile([P, SUB * K], u32, tag="io")
            # all 4 subtiles' logits into one PSUM bank [128, 256]
            ps4 = ppool.tile([P, SUB * E], f32, tag="ps")
            for u in range(SUB):
                for c in range(DC):
                    nc.tensor.matmul(
                        ps4[:, u * E : (u + 1) * E],
                        lhsT=xt[:, c * ST + u * P : c * ST + (u + 1) * P],
                        rhs=ws_sb[:, c * E : (c + 1) * E],
                        start=(c == 0),
                        stop=(c == DC - 1),
                    )
            # tail over subtile range [u0, u0+nsub): batched for steady-state
            # supertiles, per-subtile for the last one so the pipeline drains
            def tail(u0, nsub):
                lgn = work.tile([P, SUB * E], f32, tag="lg")
                lg = lgn[:, : nsub * E]
                nc.vector.tensor_add(
                    lg, ps4[:, u0 * E : (u0 + nsub) * E], bt_sb[:, : nsub * E]
                )
                v8n = small.tile([P, SUB * K], f32, tag="v8")
                for j in range(nsub):
                    nc.vector.max(
                        out=v8n[:, j * K : (j + 1) * K],
                        in_=lg[:, j * E : (j + 1) * E],
                    )
                # shared per-row shift keeps every exp argument in [-x, 0];
                # any per-row constant cancels in the softmax ratio
                nm = small.tile([P, 1], f32, tag="nm")
                nc.vector.tensor_reduce(
                    nm[:], lg, axis=Axis.X, op=Alu.max, negate=True
                )
                for j in range(nsub):
                    nc.vector.max_index(
                        io[:, (u0 + j) * K : (u0 + j + 1) * K],
                        v8n[:, j * K : (j + 1) * K],
                        lg[:, j * E : (j + 1) * E],
                    )
                # denominators: sum over each subtile's top-8 exps
                e8n = small.tile([P, SUB * K], f32, tag="e8")
                nc.scalar.activation(
                    e8n[:, : nsub * K], v8n[:, : nsub * K], Act.Exp, bias=nm[:]
                )
                s8n = small.tile([P, SUB], f32, tag="s8")
                nc.vector.tensor_reduce(
                    s8n[:, :nsub],
                    e8n[:, : nsub * K].rearrange("p (u k) -> p u k", k=K),
                    axis=Axis.X,
                    op=Alu.add,
                )
                rcn = small.tile([P, SUB], f32, tag="rc")
                nc.vector.reciprocal(rcn[:, :nsub], s8n[:, :nsub])
                exn = work.tile([P, SUB * E], f32, tag="ex")
                nc.scalar.activation(exn[:, : nsub * E], lg, Act.Exp, bias=nm[:])
                mkn = work.tile([P, SUB * E], f32, tag="mk")
                for j in range(nsub):
                    # (logit >= 8th-max) * 1/sum8  in one op
                    nc.vector.tensor_scalar(
                        out=mkn[:, j * E : (j + 1) * E],
                        in0=lg[:, j * E : (j + 1) * E],
                        scalar1=v8n[:, j * K + K - 1 : j * K + K],
                        scalar2=rcn[:, j : j + 1],
                        op0=Alu.is_ge,
                        op1=Alu.mult,
                    )
                nc.vector.tensor_mul(
                    po[:, u0 * E : (u0 + nsub) * E],
                    exn[:, : nsub * E],
                    mkn[:, : nsub * E],
                )

            if s < NST - 1:
                tail(0, SUB)
                # SWDGE (gpsimd) queues: keeps stores off the SP sequencer so
                # the next supertile's x load is never head-of-line blocked.
                nc.gpsimd.dma_start(praw[s * P : (s + 1) * P, :], po[:])
                nc.gpsimd.dma_start(iraw[s * P : (s + 1) * P, :], io[:])
            else:
                # drain the pipeline in halves; SP + HWDGE are idle by now
                h = SUB // 2
                tail(0, h)
                nc.sync.dma_start(praw[s * P : (s + 1) * P, : h * E], po[:, : h * E])
                nc.sync.dma_start(iraw[s * P : (s + 1) * P, : h * K], io[:, : h * K])
                tail(h, SUB - h)
                nc.sync.dma_start(praw[s * P : (s + 1) * P, h * E :], po[:, h * E :])
                nc.sync.dma_start(iraw[s * P : (s + 1) * P, h * K :], io[:, h * K :])

    nc.compile()
    return nc


_NC_CACHE = {}


def _get_nc(T=4096):
    if T not in _NC_CACHE:
        _NC_CACHE[T] = build_nc(T)
    return _NC_CACHE[T]


def _stage_inputs(x, W, b):
    x = np.asarray(x, dtype=np.float32)
    W = np.asarray(W, dtype=np.float32)
    b = np.asarray(b, dtype=np.float32)
    B, S, d = x.shape
    assert d == D and W.shape == (E, D) and b.shape == (E,)
    NST = S // ST
    # ws[p, c*E+e] = W[e, 128c+p]
    ws = np.ascontiguousarray(W.T.reshape(DC, P, E).transpose(1, 0, 2).reshape(P, DC * E))
    bt = np.ascontiguousarray(np.broadcast_to(np.tile(b, SUB)[None, :], (P, SUB * E)))
    in_maps = []
    for i in range(N_CORES):
        # xs[s*128+p, c*ST+t] = x[i, s*ST+t, 128c+p]
        xi = x[i].reshape(NST, ST, DC, P)  # [s, t, c, p]
        xsi = np.ascontiguousarray(xi.transpose(0, 3, 2, 1).reshape(NST * P, DC * ST))
        in_maps.append({"xs": xsi, "ws": ws, "bt": bt})
    return in_maps


def _unstage_outputs(res, S):
    NST = S // ST
    probs = np.empty((N_CORES, S, E), dtype=np.float32)
    indices = np.empty((N_CORES, S, K), dtype=np.int32)
    for i in range(N_CORES):
        pr = res[i]["praw"].reshape(NST, P, SUB, E).transpose(0, 2, 1, 3)
        probs[i] = pr.reshape(S, E)
        ir = res[i]["iraw"].view(np.int32).reshape(NST, P, SUB, K).transpose(0, 2, 1, 3)
        indices[i] = ir.reshape(S, K)
    return probs, indices


def kernel(x, W, b):
    from concourse.bass_utils import run_bass_kernel_spmd

    x = np.asarray(x, dtype=np.float32)
    B, S, d = x.shape
    assert B == N_CORES
    nc = _get_nc(S)
    in_maps = _stage_inputs(x, W, b)
    res = run_bass_kernel_spmd(nc, in_maps, list(range(N_CORES))).results
    return _unstage_outputs(res, S)


# revision 30
# speedup vs baseline: 1.0152x; 1.0110x over previous
"""NoisyTopkRouter Trainium2 kernel.

Computes, for x [8, 4096, 1024] f32, W [64, 1024], b [64]:
  logits = x @ W.T + b            [8, 4096, 64]
  top_vals, indices = top_k(logits, 8)
  probs = softmax(scatter(-inf, indices, top_vals))   (zeros off the top-8)
Returns (probs [8,4096,64] f32, indices [8,4096,8] int32).

Sharding: data-parallel over the batch dim — core i handles x[i] (4096
tokens).  Router weights are replicated.

All DRAM tensors are staged host-side as exact SBUF images so tiles move
with single large-descriptor DMAs:
  xs [NST*128, DC*ST]  xs[s*128+p, c*ST+t] = x[s*ST+t, 128c+p]
  ws [128, DC*E]       ws[p, c*E+e]        = W[e, 128c+p]
  bt [128, SUB*E]      bias replicated across partitions and subtiles
  probs_raw [NST*128, SUB*E],  idx_raw [NST*128, SUB*K]  (unshuffled on host)

The softmax subtracts one shared per-row max (over the whole batched
tile) instead of a per-subtile max: any per-row constant cancels in the
exp(v)/sum(exp(top8)) ratio, and a single shift keeps the cross-engine
dependency chain short.
"""

import sys

try:  # the axon boot usually provides concourse already
    import concourse  # noqa: F401
except ImportError:
    sys.path.insert(0, "/opt/trn_rl_repo")

import numpy as np

D = 1024
E = 64
K = 8
P = 128
DC = D // P  # 8 d-chunks of 128
N_CORES = 8
ST = 512  # tokens per supertile (one DMA batch)
SUB = ST // P  # 128-token subtiles per supertile


def build_nc(T=4096):
    """Build + compile the per-core program for T tokens."""
    import concourse.bass as bass  # noqa: F401
    import concourse.tile as tile
    from concourse import bacc, mybir

    f32 = mybir.dt.float32
    u32 = mybir.dt.uint32
    Alu = mybir.AluOpType
    Act = mybir.ActivationFunctionType
    Axis = mybir.AxisListType

    assert T % ST == 0
    NST = T // ST

    nc = bacc.Bacc("TRN2", target_bir_lowering=False, debug=False)

    xs = nc.dram_tensor("xs", [NST * P, DC * ST], f32, kind="ExternalInput").ap()
    ws = nc.dram_tensor("ws", [P, DC * E], f32, kind="ExternalInput").ap()
    bt = nc.dram_tensor("bt", [P, SUB * E], f32, kind="ExternalInput").ap()
    praw = nc.dram_tensor("praw", [NST * P, SUB * E], f32, kind="ExternalOutput").ap()
    iraw = nc.dram_tensor("iraw", [NST * P, SUB * K], u32, kind="ExternalOutput").ap()

    from contextlib import ExitStack

    with tile.TileContext(nc) as tc, ExitStack() as ctx:
        const = ctx.enter_context(tc.tile_pool(name="const", bufs=1))
        xpool = ctx.enter_context(tc.tile_pool(name="x", bufs=4))
        ppool = ctx.enter_context(tc.tile_pool(name="ps", bufs=8, space="PSUM"))
        opool = ctx.enter_context(tc.tile_pool(name="out", bufs=3))
        work = ctx.enter_context(tc.tile_pool(name="work", bufs=3))
        small = ctx.enter_context(tc.tile_pool(name="small", bufs=4))

        ws_sb = const.tile([P, DC * E], f32)
        bt_sb = const.tile([P, SUB * E], f32)

        for s in range(NST):
            # supertile in 4 token-quarter DMAs so subtile u's matmuls can
            # start as soon as its 512KB lands (512B runs: full DMA rate)
            xt = xpool.tile([P, DC * ST], f32, tag="xst")
            xt3 = xt[:].rearrange("p (c t) -> p c t", t=ST)
            xs3 = xs[s * P : (s + 1) * P, :].rearrange("p (c t) -> p c t", t=ST)
            for u in range(SUB):
                nc.sync.dma_start(
                    xt3[:, :, u * P : (u + 1) * P], xs3[:, :, u * P : (u + 1) * P]
                )
            if s == 0:
                # const loads AFTER the first x quarters: the x stream owns
                # the DMA engines from t=0; ws still lands before the first
                # matmul group needs it
                nc.sync.dma_start(ws_sb[:], ws[:])
                nc.sync.dma_start(bt_sb[:], bt[:])
            po = opool.tile([P, SUB * E], f32, tag="po")
            io = opool.tile([P, SUB * K], u32, tag="io")
            # all 4 subtiles' logits into one PSUM bank [128, 256]
            ps4 = ppool.tile([P, SUB * E], f32, tag="ps")
            for u in range(SUB):
                for c in range(DC):
                    nc.tensor.matmul(
                        ps4[:, u * E : (u + 1) * E],
                        lhsT=xt[:, c * ST + u * P : c * ST + (u + 1) * P],
                        rhs=ws_sb[:, c * E : (c + 1) * E],
                        start=(c == 0),
                        stop=(c == DC - 1),
                    )
            # tail over subtile range [u0, u0+nsub): batched for steady-state
            # supertiles, per-subtile for the last one so the pipeline drains
            def tail(u0, nsub):
                lgn = work.tile([P, SUB * E], f32, tag="lg")
                lg = lgn[:, : nsub * E]
                nc.vector.tensor_add(
                    lg, ps4[:, u0 * E : (u0 + nsub) * E], bt_sb[:, : nsub * E]
                )
                v8n = small.tile([P, SUB * K], f32, tag="v8")
                for j in range(nsub):
                    nc.vector.max(
                        out=v8n[:, j * K : (j + 1) * K],
                        in_=lg[:, j * E : (j + 1) * E],
                    )
                # shared per-row shift keeps every exp argument in [-x, 0];
                # any per-row constant cancels in the softmax ratio
                nm = small.tile([P, 1], f32, tag="nm")
                nc.vector.tensor_reduce(
                    nm[:], lg, axis=Axis.X, op=Alu.max, negate=True
                )
                for j in range(nsub):
                    nc.vector.max_index(
                        io[:, (u0 + j) * K : (u0 + j + 1) * K],
                        v8n[:, j * K : (j + 1) * K],
                        lg[:, j * E : (j + 1) * E],
                    )
                # denominators: sum over each subtile's top-8 exps
                e8n = small.tile([P, SUB * K], f32, tag="e8")
                nc.scalar.activation(
                    e8n[:, : nsub * K], v8n[:, : nsub * K], Act.Exp, bias=nm[:]
                )
                s8n = small.tile([P, SUB], f32, tag="s8")
                nc.vector.tensor_reduce(
                    s8n[:, :nsub],
                    e8n[:, : nsub * K].rearrange("p (u k) -> p u k", k=K),
                    axis=Axis.X,
                    op=Alu.add,
                )
                rcn = small.tile([P, SUB], f32, tag="rc")
                nc.vector.reciprocal(rcn[:, :nsub], s8n[:, :nsub])
                exn = work.tile([P, SUB * E], f32, tag="ex")
                nc.scalar.activation(exn[:, : nsub * E], lg, Act.Exp, bias=nm[:])
                mkn = work.tile([P, SUB * E], f32, tag="mk")
                for j in range(nsub):
                    # (logit >= 8th-max) * 1/sum8  in one op
                    nc.vector.tensor_scalar(
                        out=mkn[:, j * E : (j + 1) * E],
                        in0=lg[:, j * E : (j + 1) * E],
                        scalar1=v8n[:, j * K + K - 1 : j * K + K],
                        scalar2=rcn[:, j : j + 1],
                        op0=Alu.is_ge,
                        op1=Alu.mult,
                    )
                nc.vector.tensor_mul(
                    po[:, u0 * E : (u0 + nsub) * E],
                    exn[:, : nsub * E],
                    mkn[:, : nsub * E],
                )

            if s < NST - 1:
                tail(0, SUB)
                # SWDGE (gpsimd) queues: keeps stores off the SP sequencer so
                # the next supertile's x load is never head-of-line blocked.
                nc.gpsimd.dma_start(praw[s * P : (s + 1) * P, :], po[:])
                nc.gpsimd.dma_start(iraw[s * P : (s + 1) * P, :], io[:])
            else:
                # drain the pipeline in halves; SP + HWDGE are idle by now
                h = SUB // 2
                tail(0, h)
                nc.sync.dma_start(praw[s * P : (s + 1) * P, : h * E], po[:, : h * E])
                nc.sync.dma_start(iraw[s * P : (s + 1) * P, : h * K], io[:, : h * K])
                tail(h, SUB - h)
                nc.sync.dma_start(praw[s * P : (s + 1) * P, h * E :], po[:, h * E :])
                nc.sync.dma_start(iraw[s * P : (s + 1) * P, h * K :], io[:, h * K :])

    nc.compile()
    return nc


_NC_CACHE = {}


def _get_nc(T=4096):
    if T not in _NC_CACHE:
        _NC_CACHE[T] = build_nc(T)
    return _NC_CACHE[T]


def _stage_inputs(x, W, b):
    x = np.asarray(x, dtype=np.float32)
    W = np.asarray(W, dtype=np.float32)
    b = np.asarray(b, dtype=np.float32)
    B, S, d = x.shape
    assert d == D and W.shape == (E, D) and b.shape == (E,)
    NST = S // ST
    # ws[p, c*E+e] = W[e, 128c+p]
    ws = np.ascontiguousarray(W.T.reshape(DC, P, E).transpose(1, 0, 2).reshape(P, DC * E))
    bt = np.ascontiguousarray(np.broadcast_to(np.tile(b, SUB)[None, :], (P, SUB * E)))
    in_maps = []
    for i in range(N_CORES):
        # xs[s*128+p, c*ST+t] = x[i, s*ST+t, 128c+p]
        xi = x[i].reshape(NST, ST, DC, P)  # [s, t, c, p]
        xsi = np.ascontiguousarray(xi.transpose(0, 3, 2, 1).reshape(NST * P, DC * ST))
        in_maps.append({"xs": xsi, "ws": ws, "bt": bt})
    return in_maps


def _unstage_outputs(res, S):
    NST = S // ST
    probs = np.empty((N_CORES, S, E), dtype=np.float32)
    indices = np.empty((N_CORES, S, K), dtype=np.int32)
    for i in range(N_CORES):
        pr = res[i]["praw"].reshape(NST, P, SUB, E).transpose(0, 2, 1, 3)
        probs[i] = pr.reshape(S, E)
        ir = res[i]["iraw"].view(np.int32).reshape(NST, P, SUB, K).transpose(0, 2, 1, 3)
        indices[i] = ir.reshape(S, K)
    return probs, indices


def kernel(x, W, b):
    from concourse.bass_utils import run_bass_kernel_spmd

    x = np.asarray(x, dtype=np.float32)
    B, S, d = x.shape
    assert B == N_CORES
    nc = _get_nc(S)
    in_maps = _stage_inputs(x, W, b)
    res = run_bass_kernel_spmd(nc, in_maps, list(range(N_CORES))).results
    return _unstage_outputs(res, S)


# revision 34
# speedup vs baseline: 1.0355x; 1.0199x over previous
"""NoisyTopkRouter Trainium2 kernel.

Computes, for x [8, 4096, 1024] f32, W [64, 1024], b [64]:
  logits = x @ W.T + b            [8, 4096, 64]
  top_vals, indices = top_k(logits, 8)
  probs = softmax(scatter(-inf, indices, top_vals))   (zeros off the top-8)
Returns (probs [8,4096,64] f32, indices [8,4096,8] int32).

Sharding: data-parallel over the batch dim — core i handles x[i] (4096
tokens).  Router weights are replicated.

All DRAM tensors are staged host-side as exact SBUF images so tiles move
with single large-descriptor DMAs:
  xs [NST*128, DC*ST]  xs[s*128+p, c*ST+t] = x[s*ST+t, 128c+p]
  ws [128, DC*E]       ws[p, c*E+e]        = W[e, 128c+p]
  bt [128, SUB*E]      bias replicated across partitions and subtiles
  probs_raw [NST*128, SUB*E],  idx_raw [NST*128, SUB*K]  (unshuffled on host)

The softmax subtracts one shared per-row max (over the whole batched
tile) instead of a per-subtile max: any per-row constant cancels in the
exp(v)/sum(exp(top8)) ratio, and a single shift keeps the cross-engine
dependency chain short.
"""

import sys

try:  # the axon boot usually provides concourse already
    import concourse  # noqa: F401
except ImportError:
    sys.path.insert(0, "/opt/trn_rl_repo")

import numpy as np

D = 1024
E = 64
K = 8
P = 128
DC = D // P  # 8 d-chunks of 128
N_CORES = 8
ST = 512  # tokens per supertile (one DMA batch)
SUB = ST // P  # 128-token subtiles per supertile


def build_nc(T=4096):
    """Build + compile the per-core program for T tokens."""
    import concourse.bass as bass  # noqa: F401
    import concourse.tile as tile
    from concourse import bacc, mybir

    f32 = mybir.dt.float32
    u32 = mybir.dt.uint32
    Alu = mybir.AluOpType
    Act = mybir.ActivationFunctionType
    Axis = mybir.AxisListType

    assert T % ST == 0
    NST = T // ST

    nc = bacc.Bacc("TRN2", target_bir_lowering=False, debug=False)

    xs = nc.dram_tensor("xs", [NST * P, DC * ST], f32, kind="ExternalInput").ap()
    ws = nc.dram_tensor("ws", [P, DC * E], f32, kind="ExternalInput").ap()
    bt = nc.dram_tensor("bt", [P, SUB * E], f32, kind="ExternalInput").ap()
    praw = nc.dram_tensor("praw", [NST * P, SUB * E], f32, kind="ExternalOutput").ap()
    iraw = nc.dram_tensor("iraw", [NST * P, SUB * K], u32, kind="ExternalOutput").ap()

    from contextlib import ExitStack

    with tile.TileContext(nc) as tc, ExitStack() as ctx:
        const = ctx.enter_context(tc.tile_pool(name="const", bufs=1))
        xpool = ctx.enter_context(tc.tile_pool(name="x", bufs=4))
        ppool = ctx.enter_context(tc.tile_pool(name="ps", bufs=4, space="PSUM"))
        opool = ctx.enter_context(tc.tile_pool(name="out", bufs=3))
        work = ctx.enter_context(tc.tile_pool(name="work", bufs=3))
        small = ctx.enter_context(tc.tile_pool(name="small", bufs=4))

        ws_sb = const.tile([P, DC * E], f32)
        bt_sb = const.tile([P, SUB * E], f32)

        for s in range(NST):
            # supertile in 4 token-quarter DMAs so subtile u's matmuls can
            # start as soon as its 512KB lands (512B runs: full DMA rate)
            xt = xpool.tile([P, DC * ST], f32, tag="xst")
            xt3 = xt[:].rearrange("p (c t) -> p c t", t=ST)
            xs3 = xs[s * P : (s + 1) * P, :].rearrange("p (c t) -> p c t", t=ST)
            for u in range(SUB):
                nc.sync.dma_start(
                    xt3[:, :, u * P : (u + 1) * P], xs3[:, :, u * P : (u + 1) * P]
                )
            if s == 0:
                # const loads AFTER the first x quarters: the x stream owns
                # the DMA engines from t=0; ws still lands before the first
                # matmul group needs it
                nc.sync.dma_start(ws_sb[:], ws[:])
                nc.sync.dma_start(bt_sb[:], bt[:])
            po = opool.tile([P, SUB * E], f32, tag="po")
            io = opool.tile([P, SUB * K], u32, tag="io")
            # two PSUM tiles (= two banks) per supertile: PSUM hazards are
            # tracked bank-level, so one shared bank would serialize the
            # first half-tail behind the last subtile's matmuls
            HB = SUB // 2  # subtiles per half
            psA = ppool.tile([P, HB * E], f32, tag="psA")
            psB = ppool.tile([P, HB * E], f32, tag="psB")
            psh = [psA, psB]
            for u in range(SUB):
                for c in range(DC):
                    nc.tensor.matmul(
                        psh[u // HB][:, (u % HB) * E : (u % HB + 1) * E],
                        lhsT=xt[:, c * ST + u * P : c * ST + (u + 1) * P],
                        rhs=ws_sb[:, c * E : (c + 1) * E],
                        start=(c == 0),
                        stop=(c == DC - 1),
                    )
            # tail over subtile range [u0, u0+nsub): batched for steady-state
            # supertiles, per-subtile for the last one so the pipeline drains
            def tail(u0, nsub):
                lgn = work.tile([P, SUB * E], f32, tag="lg")
                lg = lgn[:, : nsub * E]
                # bias-add per PSUM half so each half-tail gates only on its
                # own bank's matmuls
                for h0 in range(u0, u0 + nsub, HB):
                    n = min(HB, u0 + nsub - h0)
                    nc.vector.tensor_add(
                        lg[:, (h0 - u0) * E : (h0 - u0 + n) * E],
                        psh[h0 // HB][:, (h0 % HB) * E : (h0 % HB + n) * E],
                        bt_sb[:, : n * E],
                    )
                v8n = small.tile([P, SUB * K], f32, tag="v8")
                for j in range(nsub):
                    nc.vector.max(
                        out=v8n[:, j * K : (j + 1) * K],
                        in_=lg[:, j * E : (j + 1) * E],
                    )
                # shared per-row shift keeps every exp argument in [-x, 0];
                # any per-row constant cancels in the softmax ratio
                nm = small.tile([P, 1], f32, tag="nm")
                nc.vector.tensor_reduce(
                    nm[:], lg, axis=Axis.X, op=Alu.max, negate=True
                )
                for j in range(nsub):
                    nc.vector.max_index(
                        io[:, (u0 + j) * K : (u0 + j + 1) * K],
                        v8n[:, j * K : (j + 1) * K],
                        lg[:, j * E : (j + 1) * E],
                    )
                # denominators: sum over each subtile's top-8 exps
                e8n = small.tile([P, SUB * K], f32, tag="e8")
                nc.scalar.activation(
                    e8n[:, : nsub * K], v8n[:, : nsub * K], Act.Exp, bias=nm[:]
                )
                s8n = small.tile([P, SUB], f32, tag="s8")
                nc.vector.tensor_reduce(
                    s8n[:, :nsub],
                    e8n[:, : nsub * K].rearrange("p (u k) -> p u k", k=K),
                    axis=Axis.X,
                    op=Alu.add,
                )
                rcn = small.tile([P, SUB], f32, tag="rc")
                nc.vector.reciprocal(rcn[:, :nsub], s8n[:, :nsub])
                exn = work.tile([P, SUB * E], f32, tag="ex")
                nc.scalar.activation(exn[:, : nsub * E], lg, Act.Exp, bias=nm[:])
                mkn = work.tile([P, SUB * E], f32, tag="mk")
                for j in range(nsub):
                    # (logit >= 8th-max) * 1/sum8  in one op
                    nc.vector.tensor_scalar(
                        out=mkn[:, j * E : (j + 1) * E],
                        in0=lg[:, j * E : (j + 1) * E],
                        scalar1=v8n[:, j * K + K - 1 : j * K + K],
                        scalar2=rcn[:, j : j + 1],
                        op0=Alu.is_ge,
                        op1=Alu.mult,
                    )
                nc.vector.tensor_mul(
                    po[:, u0 * E : (u0 + nsub) * E],
                    exn[:, : nsub * E],
                    mkn[:, : nsub * E],
                )

            if s < NST - 1:
                tail(0, SUB)
                # SWDGE (gpsimd) queues: keeps stores off the SP sequencer so
                # the next supertile's x load is never head-of-line blocked.
                nc.gpsimd.dma_start(praw[s * P : (s + 1) * P, :], po[:])
                nc.gpsimd.dma_start(iraw[s * P : (s + 1) * P, :], io[:])
            else:
                # drain the pipeline in halves; SP + HWDGE are idle by now
                h = SUB // 2
                tail(0, h)
                nc.sync.dma_start(praw[s * P : (s + 1) * P, : h * E], po[:, : h * E])
                nc.sync.dma_start(iraw[s * P : (s + 1) * P, : h * K], io[:, : h * K])
                tail(h, SUB - h)
                nc.sync.dma_start(praw[s * P : (s + 1) * P, h * E :], po[:, h * E :])
                nc.sync.dma_start(iraw[s * P : (s + 1) * P, h * K :], io[:, h * K :])

    nc.compile()
    return nc


_NC_CACHE = {}


def _get_nc(T=4096):
    if T not in _NC_CACHE:
        _NC_CACHE[T] = build_nc(T)
    return _NC_CACHE[T]


def _stage_inputs(x, W, b):
    x = np.asarray(x, dtype=np.float32)
    W = np.asarray(W, dtype=np.float32)
    b = np.asarray(b, dtype=np.float32)
    B, S, d = x.shape
    assert d == D and W.shape == (E, D) and b.shape == (E,)
    NST = S // ST
    # ws[p, c*E+e] = W[e, 128c+p]
    ws = np.ascontiguousarray(W.T.reshape(DC, P, E).transpose(1, 0, 2).reshape(P, DC * E))
    bt = np.ascontiguousarray(np.broadcast_to(np.tile(b, SUB)[None, :], (P, SUB * E)))
    in_maps = []
    for i in range(N_CORES):
        # xs[s*128+p, c*ST+t] = x[i, s*ST+t, 128c+p]
        xi = x[i].reshape(NST, ST, DC, P)  # [s, t, c, p]
        xsi = np.ascontiguousarray(xi.transpose(0, 3, 2, 1).reshape(NST * P, DC * ST))
        in_maps.append({"xs": xsi, "ws": ws, "bt": bt})
    return in_maps


def _unstage_outputs(res, S):
    NST = S // ST
    probs = np.empty((N_CORES, S, E), dtype=np.float32)
    indices = np.empty((N_CORES, S, K), dtype=np.int32)
    for i in range(N_CORES):
        pr = res[i]["praw"].reshape(NST, P, SUB, E).transpose(0, 2, 1, 3)
        probs[i] = pr.reshape(S, E)
        ir = res[i]["iraw"].view(np.int32).reshape(NST, P, SUB, K).transpose(0, 2, 1, 3)
        indices[i] = ir.reshape(S, K)
    return probs, indices


def kernel(x, W, b):
    from concourse.bass_utils import run_bass_kernel_spmd

    x = np.asarray(x, dtype=np.float32)
    B, S, d = x.shape
    assert B == N_CORES
    nc = _get_nc(S)
    in_maps = _stage_inputs(x, W, b)
    res = run_bass_kernel_spmd(nc, in_maps, list(range(N_CORES))).results
    return _unstage_outputs(res, S)


# revision 40
# speedup vs baseline: 1.0709x; 1.0342x over previous
"""NoisyTopkRouter Trainium2 kernel.

Computes, for x [8, 4096, 1024] f32, W [64, 1024], b [64]:
  logits = x @ W.T + b            [8, 4096, 64]
  top_vals, indices = top_k(logits, 8)
  probs = softmax(scatter(-inf, indices, top_vals))   (zeros off the top-8)
Returns (probs [8,4096,64] f32, indices [8,4096,8] int32).

Sharding: data-parallel over the batch dim — core i handles x[i] (4096
tokens).  Router weights are replicated.

All DRAM tensors are staged host-side as exact SBUF images so tiles move
with single large-descriptor DMAs:
  xs [NST*128, DC*ST]  xs[s*128+p, c*ST+t] = x[s*ST+t, 128c+p]
  ws [128, DC*E]       ws[p, c*E+e]        = W[e, 128c+p]
  bt [128, SUB*E]      bias replicated across partitions and subtiles
  probs_raw [NST*128, SUB*E],  idx_raw [NST*128, SUB*K]  (unshuffled on host)

The softmax subtracts one shared per-row max (over the whole batched
tile) instead of a per-subtile max: any per-row constant cancels in the
exp(v)/sum(exp(top8)) ratio, and a single shift keeps the cross-engine
dependency chain short.
"""

import sys

try:  # the axon boot usually provides concourse already
    import concourse  # noqa: F401
except ImportError:
    sys.path.insert(0, "/opt/trn_rl_repo")

import numpy as np

D = 1024
E = 64
K = 8
P = 128
DC = D // P  # 8 d-chunks of 128
N_CORES = 8
ST = 512  # tokens per supertile (one DMA batch)
SUB = ST // P  # 128-token subtiles per supertile


def build_nc(T=4096):
    """Build + compile the per-core program for T tokens."""
    import concourse.bass as bass  # noqa: F401
    import concourse.tile as tile
    from concourse import bacc, mybir

    f32 = mybir.dt.float32
    u32 = mybir.dt.uint32
    Alu = mybir.AluOpType
    Act = mybir.ActivationFunctionType
    Axis = mybir.AxisListType

    assert T % ST == 0
    NST = T // ST

    nc = bacc.Bacc("TRN2", target_bir_lowering=False, debug=False)

    xs = nc.dram_tensor("xs", [NST * P, DC * ST], f32, kind="ExternalInput").ap()
    ws = nc.dram_tensor("ws", [P, DC * E], f32, kind="ExternalInput").ap()
    bt = nc.dram_tensor("bt", [P, SUB * E], f32, kind="ExternalInput").ap()
    praw = nc.dram_tensor("praw", [NST * P, SUB * K], f32, kind="ExternalOutput").ap()
    iraw = nc.dram_tensor("iraw", [NST * P, SUB * K], u32, kind="ExternalOutput").ap()

    from contextlib import ExitStack

    with tile.TileContext(nc) as tc, ExitStack() as ctx:
        const = ctx.enter_context(tc.tile_pool(name="const", bufs=1))
        xpool = ctx.enter_context(tc.tile_pool(name="x", bufs=4))
        ppool = ctx.enter_context(tc.tile_pool(name="ps", bufs=4, space="PSUM"))
        opool = ctx.enter_context(tc.tile_pool(name="out", bufs=3))
        work = ctx.enter_context(tc.tile_pool(name="work", bufs=3))
        small = ctx.enter_context(tc.tile_pool(name="small", bufs=4))

        ws_sb = const.tile([P, DC * E], f32)
        bt_sb = const.tile([P, SUB * E], f32)

        for s in range(NST):
            # supertile in 4 token-quarter DMAs so subtile u's matmuls can
            # start as soon as its 512KB lands (512B runs: full DMA rate)
            xt = xpool.tile([P, DC * ST], f32, tag="xst")
            xt3 = xt[:].rearrange("p (c t) -> p c t", t=ST)
            xs3 = xs[s * P : (s + 1) * P, :].rearrange("p (c t) -> p c t", t=ST)
            for u in range(SUB):
                nc.sync.dma_start(
                    xt3[:, :, u * P : (u + 1) * P], xs3[:, :, u * P : (u + 1) * P]
                )
            if s == 0:
                # const loads AFTER the first x quarters: the x stream owns
                # the DMA engines from t=0; ws still lands before the first
                # matmul group needs it
                nc.sync.dma_start(ws_sb[:], ws[:])
                nc.sync.dma_start(bt_sb[:], bt[:])
            po = opool.tile([P, SUB * K], f32, tag="po")
            io = opool.tile([P, SUB * K], u32, tag="io")
            # two PSUM tiles (= two banks) per supertile: PSUM hazards are
            # tracked bank-level, so one shared bank would serialize the
            # first half-tail behind the last subtile's matmuls
            HB = SUB // 2  # subtiles per half
            psA = ppool.tile([P, HB * E], f32, tag="psA")
            psB = ppool.tile([P, HB * E], f32, tag="psB")
            psh = [psA, psB]
            for u in range(SUB):
                for c in range(DC):
                    nc.tensor.matmul(
                        psh[u // HB][:, (u % HB) * E : (u % HB + 1) * E],
                        lhsT=xt[:, c * ST + u * P : c * ST + (u + 1) * P],
                        rhs=ws_sb[:, c * E : (c + 1) * E],
                        start=(c == 0),
                        stop=(c == DC - 1),
                    )
            # tail over subtile range [u0, u0+nsub): batched for steady-state
            # supertiles, per-subtile for the last one so the pipeline drains
            def tail(u0, nsub):
                lgn = work.tile([P, SUB * E], f32, tag="lg")
                lg = lgn[:, : nsub * E]
                # bias-add per PSUM half so each half-tail gates only on its
                # own bank's matmuls
                for h0 in range(u0, u0 + nsub, HB):
                    n = min(HB, u0 + nsub - h0)
                    nc.vector.tensor_add(
                        lg[:, (h0 - u0) * E : (h0 - u0 + n) * E],
                        psh[h0 // HB][:, (h0 % HB) * E : (h0 % HB + n) * E],
                        bt_sb[:, : n * E],
                    )
                v8n = small.tile([P, SUB * K], f32, tag="v8")
                for j in range(nsub):
                    nc.vector.max(
                        out=v8n[:, j * K : (j + 1) * K],
                        in_=lg[:, j * E : (j + 1) * E],
                    )
                # shared per-row shift keeps every exp argument in [-x, 0];
                # any per-row constant cancels in the softmax ratio
                nm = small.tile([P, 1], f32, tag="nm")
                nc.vector.tensor_reduce(
                    nm[:], lg, axis=Axis.X, op=Alu.max, negate=True
                )
                for j in range(nsub):
                    nc.vector.max_index(
                        io[:, (u0 + j) * K : (u0 + j + 1) * K],
                        v8n[:, j * K : (j + 1) * K],
                        lg[:, j * E : (j + 1) * E],
                    )
                # denominators: sum over each subtile's top-8 exps
                e8n = small.tile([P, SUB * K], f32, tag="e8")
                nc.scalar.activation(
                    e8n[:, : nsub * K], v8n[:, : nsub * K], Act.Exp, bias=nm[:]
                )
                s8n = small.tile([P, SUB], f32, tag="s8")
                nc.vector.tensor_reduce(
                    s8n[:, :nsub],
                    e8n[:, : nsub * K].rearrange("p (u k) -> p u k", k=K),
                    axis=Axis.X,
                    op=Alu.add,
                )
                rcn = small.tile([P, SUB], f32, tag="rc")
                nc.vector.reciprocal(rcn[:, :nsub], s8n[:, :nsub])
                # compact output: only the top-8 probs, p8 = exp(v8-m)/sum8;
                # the dense [T,64] tensor is scattered on the host
                for j in range(nsub):
                    nc.vector.tensor_scalar(
                        out=po[:, (u0 + j) * K : (u0 + j + 1) * K],
                        in0=e8n[:, j * K : (j + 1) * K],
                        scalar1=rcn[:, j : j + 1],
                        scalar2=None,
                        op0=Alu.mult,
                    )

            if s < NST - 1:
                tail(0, SUB)
                # SWDGE (gpsimd) queues: keeps stores off the SP sequencer so
                # the next supertile's x load is never head-of-line blocked.
                nc.gpsimd.dma_start(praw[s * P : (s + 1) * P, :], po[:])
                nc.gpsimd.dma_start(iraw[s * P : (s + 1) * P, :], io[:])
            else:
                # drain the pipeline in halves; SP + HWDGE are idle by now
                h = SUB // 2
                tail(0, h)
                nc.sync.dma_start(praw[s * P : (s + 1) * P, : h * K], po[:, : h * K])
                nc.sync.dma_start(iraw[s * P : (s + 1) * P, : h * K], io[:, : h * K])
                tail(h, SUB - h)
                nc.sync.dma_start(praw[s * P : (s + 1) * P, h * K :], po[:, h * K :])
                nc.sync.dma_start(iraw[s * P : (s + 1) * P, h * K :], io[:, h * K :])

    nc.compile()
    return nc


_NC_CACHE = {}


def _get_nc(T=4096):
    if T not in _NC_CACHE:
        _NC_CACHE[T] = build_nc(T)
    return _NC_CACHE[T]


def _stage_inputs(x, W, b):
    x = np.asarray(x, dtype=np.float32)
    W = np.asarray(W, dtype=np.float32)
    b = np.asarray(b, dtype=np.float32)
    B, S, d = x.shape
    assert d == D and W.shape == (E, D) and b.shape == (E,)
    NST = S // ST
    # ws[p, c*E+e] = W[e, 128c+p]
    ws = np.ascontiguousarray(W.T.reshape(DC, P, E).transpose(1, 0, 2).reshape(P, DC * E))
    bt = np.ascontiguousarray(np.broadcast_to(np.tile(b, SUB)[None, :], (P, SUB * E)))
    in_maps = []
    for i in range(N_CORES):
        # xs[s*128+p, c*ST+t] = x[i, s*ST+t, 128c+p]
        xi = x[i].reshape(NST, ST, DC, P)  # [s, t, c, p]
        xsi = np.ascontiguousarray(xi.transpose(0, 3, 2, 1).reshape(NST * P, DC * ST))
        in_maps.append({"xs": xsi, "ws": ws, "bt": bt})
    return in_maps


def _unstage_outputs(res, S):
    NST = S // ST
    probs = np.empty((N_CORES, S, E), dtype=np.float32)
    indices = np.empty((N_CORES, S, K), dtype=np.int32)
    for i in range(N_CORES):
        p8 = res[i]["praw"].reshape(NST, P, SUB, K).transpose(0, 2, 1, 3).reshape(S, K)
        ir = res[i]["iraw"].view(np.int32).reshape(NST, P, SUB, K).transpose(0, 2, 1, 3)
        indices[i] = ir.reshape(S, K)
        probs[i] = 0.0
        np.put_along_axis(probs[i], indices[i], p8, axis=1)
    return probs, indices


def kernel(x, W, b):
    from concourse.bass_utils import run_bass_kernel_spmd

    x = np.asarray(x, dtype=np.float32)
    B, S, d = x.shape
    assert B == N_CORES
    nc = _get_nc(S)
    in_maps = _stage_inputs(x, W, b)
    res = run_bass_kernel_spmd(nc, in_maps, list(range(N_CORES))).results
    return _unstage_outputs(res, S)


# revision 44
# speedup vs baseline: 1.0782x; 1.0068x over previous
"""NoisyTopkRouter Trainium2 kernel.

Computes, for x [8, 4096, 1024] f32, W [64, 1024], b [64]:
  logits = x @ W.T + b            [8, 4096, 64]
  top_vals, indices = top_k(logits, 8)
  probs = softmax(scatter(-inf, indices, top_vals))   (zeros off the top-8)
Returns (probs [8,4096,64] f32, indices [8,4096,8] int32).

Sharding: data-parallel over the batch dim — core i handles x[i] (4096
tokens).  Router weights are replicated.

All DRAM tensors are staged host-side as exact SBUF images so tiles move
with single large-descriptor DMAs:
  xs [NST*128, DC*ST]  xs[s*128+p, c*ST+t] = x[s*ST+t, 128c+p]
  ws [128, DC*E]       ws[p, c*E+e]        = W[e, 128c+p]
  bt [128, SUB*E]      bias replicated across partitions and subtiles
  p8_raw [NST*128, SUB*K], idx_raw [NST*128, SUB*K]  (device returns only
  the top-8 probs; host unstages by scattering them into the dense [T,64])

The softmax subtracts one shared per-row max (over the whole batched
tile) instead of a per-subtile max: any per-row constant cancels in the
exp(v)/sum(exp(top8)) ratio, and a single shift keeps the cross-engine
dependency chain short.
"""

import sys

try:  # the axon boot usually provides concourse already
    import concourse  # noqa: F401
except ImportError:
    sys.path.insert(0, "/opt/trn_rl_repo")

import numpy as np

D = 1024
E = 64
K = 8
P = 128
DC = D // P  # 8 d-chunks of 128
N_CORES = 8
ST = 512  # tokens per supertile (one DMA batch)
SUB = ST // P  # 128-token subtiles per supertile


def build_nc(T=4096):
    """Build + compile the per-core program for T tokens."""
    import concourse.bass as bass  # noqa: F401
    import concourse.tile as tile
    from concourse import bacc, mybir

    f32 = mybir.dt.float32
    u32 = mybir.dt.uint32
    Alu = mybir.AluOpType
    Act = mybir.ActivationFunctionType
    Axis = mybir.AxisListType

    assert T % ST == 0
    NST = T // ST

    nc = bacc.Bacc("TRN2", target_bir_lowering=False, debug=False)

    xs = nc.dram_tensor("xs", [NST * P, DC * ST], f32, kind="ExternalInput").ap()
    ws = nc.dram_tensor("ws", [P, DC * E], f32, kind="ExternalInput").ap()
    bt = nc.dram_tensor("bt", [P, SUB * E], f32, kind="ExternalInput").ap()
    praw = nc.dram_tensor("praw", [NST * P, SUB * K], f32, kind="ExternalOutput").ap()
    iraw = nc.dram_tensor("iraw", [NST * P, SUB * K], u32, kind="ExternalOutput").ap()

    from contextlib import ExitStack

    with tile.TileContext(nc) as tc, ExitStack() as ctx:
        const = ctx.enter_context(tc.tile_pool(name="const", bufs=1))
        xpool = ctx.enter_context(tc.tile_pool(name="x", bufs=4))
        ppool = ctx.enter_context(tc.tile_pool(name="ps", bufs=4, space="PSUM"))
        opool = ctx.enter_context(tc.tile_pool(name="out", bufs=3))
        work = ctx.enter_context(tc.tile_pool(name="work", bufs=3))
        small = ctx.enter_context(tc.tile_pool(name="small", bufs=4))

        ws_sb = const.tile([P, DC * E], f32)
        bt_sb = const.tile([P, SUB * E], f32)

        for s in range(NST):
            # supertile in 4 token-quarter DMAs so subtile u's matmuls can
            # start as soon as its 512KB lands (512B runs: full DMA rate)
            xt = xpool.tile([P, DC * ST], f32, tag="xst")
            xt3 = xt[:].rearrange("p (c t) -> p c t", t=ST)
            xs3 = xs[s * P : (s + 1) * P, :].rearrange("p (c t) -> p c t", t=ST)
            for u in range(SUB):
                if s == NST - 1 and u == SUB - 1:
                    # very last quarter split along d-chunks (8KB runs, full
                    # rate): the first 4 matmuls overlap the second half's
                    # transfer, starting the drain chain earlier
                    for ch in (slice(0, DC // 2), slice(DC // 2, DC)):
                        nc.sync.dma_start(
                            xt3[:, ch, u * P : (u + 1) * P],
                            xs3[:, ch, u * P : (u + 1) * P],
                        )
                else:
                    nc.sync.dma_start(
                        xt3[:, :, u * P : (u + 1) * P], xs3[:, :, u * P : (u + 1) * P]
                    )
            if s == 0:
                # const loads AFTER the first x quarters: the x stream owns
                # the DMA engines from t=0; ws still lands before the first
                # matmul group needs it
                nc.sync.dma_start(ws_sb[:], ws[:])
                nc.sync.dma_start(bt_sb[:], bt[:])
            po = opool.tile([P, SUB * K], f32, tag="po")
            io = opool.tile([P, SUB * K], u32, tag="io")
            # two PSUM tiles (= two banks) per supertile: PSUM hazards are
            # tracked bank-level, so one shared bank would serialize the
            # first half-tail behind the last subtile's matmuls
            HB = SUB // 2  # subtiles per half
            psA = ppool.tile([P, HB * E], f32, tag="psA")
            psB = ppool.tile([P, HB * E], f32, tag="psB")
            psh = [psA, psB]
            for u in range(SUB):
                for c in range(DC):
                    nc.tensor.matmul(
                        psh[u // HB][:, (u % HB) * E : (u % HB + 1) * E],
                        lhsT=xt[:, c * ST + u * P : c * ST + (u + 1) * P],
                        rhs=ws_sb[:, c * E : (c + 1) * E],
                        start=(c == 0),
                        stop=(c == DC - 1),
                    )
            # tail over subtile range [u0, u0+nsub): batched for steady-state
            # supertiles, per-subtile for the last one so the pipeline drains
            def tail(u0, nsub):
                lgn = work.tile([P, SUB * E], f32, tag="lg")
                lg = lgn[:, : nsub * E]
                # bias-add per PSUM half so each half-tail gates only on its
                # own bank's matmuls
                for h0 in range(u0, u0 + nsub, HB):
                    n = min(HB, u0 + nsub - h0)
                    nc.vector.tensor_add(
                        lg[:, (h0 - u0) * E : (h0 - u0 + n) * E],
                        psh[h0 // HB][:, (h0 % HB) * E : (h0 % HB + n) * E],
                        bt_sb[:, : n * E],
                    )
                v8n = small.tile([P, SUB * K], f32, tag="v8")
                for j in range(nsub):
                    nc.vector.max(
                        out=v8n[:, j * K : (j + 1) * K],
                        in_=lg[:, j * E : (j + 1) * E],
                    )
                # shared per-row shift keeps every exp argument in [-x, 0];
                # any per-row constant cancels in the softmax ratio
                nm = small.tile([P, 1], f32, tag="nm")
                nc.vector.tensor_reduce(
                    nm[:], lg, axis=Axis.X, op=Alu.max, negate=True
                )
                for j in range(nsub):
                    nc.vector.max_index(
                        io[:, (u0 + j) * K : (u0 + j + 1) * K],
                        v8n[:, j * K : (j + 1) * K],
                        lg[:, j * E : (j + 1) * E],
                    )
                # denominators: sum over each subtile's top-8 exps
                e8n = small.tile([P, SUB * K], f32, tag="e8")
                nc.scalar.activation(
                    e8n[:, : nsub * K], v8n[:, : nsub * K], Act.Exp, bias=nm[:]
                )
                s8n = small.tile([P, SUB], f32, tag="s8")
                nc.vector.tensor_reduce(
                    s8n[:, :nsub],
                    e8n[:, : nsub * K].rearrange("p (u k) -> p u k", k=K),
                    axis=Axis.X,
                    op=Alu.add,
                )
                rcn = small.tile([P, SUB], f32, tag="rc")
                nc.vector.reciprocal(rcn[:, :nsub], s8n[:, :nsub])
                # compact output: only the top-8 probs, p8 = exp(v8-m)/sum8;
                # the dense [T,64] tensor is scattered on the host
                for j in range(nsub):
                    nc.vector.tensor_scalar(
                        out=po[:, (u0 + j) * K : (u0 + j + 1) * K],
                        in0=e8n[:, j * K : (j + 1) * K],
                        scalar1=rcn[:, j : j + 1],
                        scalar2=None,
                        op0=Alu.mult,
                    )

            if s < NST - 1:
                tail(0, SUB)
                # SWDGE (gpsimd) queues: keeps stores off the SP sequencer so
                # the next supertile's x load is never head-of-line blocked.
                nc.gpsimd.dma_start(praw[s * P : (s + 1) * P, :], po[:])
                nc.gpsimd.dma_start(iraw[s * P : (s + 1) * P, :], io[:])
            else:
                # drain the pipeline in halves; SP + HWDGE are idle by now
                h = SUB // 2
                tail(0, h)
                nc.sync.dma_start(praw[s * P : (s + 1) * P, : h * K], po[:, : h * K])
                nc.sync.dma_start(iraw[s * P : (s + 1) * P, : h * K], io[:, : h * K])
                tail(h, SUB - h)
                nc.sync.dma_start(praw[s * P : (s + 1) * P, h * K :], po[:, h * K :])
                nc.sync.dma_start(iraw[s * P : (s + 1) * P, h * K :], io[:, h * K :])

    nc.compile()
    return nc


_NC_CACHE = {}


def _get_nc(T=4096):
    if T not in _NC_CACHE:
        _NC_CACHE[T] = build_nc(T)
    return _NC_CACHE[T]


def _stage_inputs(x, W, b):
    x = np.asarray(x, dtype=np.float32)
    W = np.asarray(W, dtype=np.float32)
    b = np.asarray(b, dtype=np.float32)
    B, S, d = x.shape
    assert d == D and W.shape == (E, D) and b.shape == (E,)
    NST = S // ST
    # ws[p, c*E+e] = W[e, 128c+p]
    ws = np.ascontiguousarray(W.T.reshape(DC, P, E).transpose(1, 0, 2).reshape(P, DC * E))
    bt = np.ascontiguousarray(np.broadcast_to(np.tile(b, SUB)[None, :], (P, SUB * E)))
    in_maps = []
    for i in range(N_CORES):
        # xs[s*128+p, c*ST+t] = x[i, s*ST+t, 128c+p]
        xi = x[i].reshape(NST, ST, DC, P)  # [s, t, c, p]
        xsi = np.ascontiguousarray(xi.transpose(0, 3, 2, 1).reshape(NST * P, DC * ST))
        in_maps.append({"xs": xsi, "ws": ws, "bt": bt})
    return in_maps


def _unstage_outputs(res, S):
    NST = S // ST
    probs = np.empty((N_CORES, S, E), dtype=np.float32)
    indices = np.empty((N_CORES, S, K), dtype=np.int32)
    for i in range(N_CORES):
        p8 = res[i]["praw"].reshape(NST, P, SUB, K).transpose(0, 2, 1, 3).reshape(S, K)
        ir = res[i]["iraw"].view(np.int32).reshape(NST, P, SUB, K).transpose(0, 2, 1, 3)
        indices[i] = ir.reshape(S, K)
        probs[i] = 0.0
        np.put_along_axis(probs[i], indices[i], p8, axis=1)
    return probs, indices


def kernel(x, W, b):
    from concourse.bass_utils import run_bass_kernel_spmd

    x = np.asarray(x, dtype=np.float32)
    B, S, d = x.shape
    assert B == N_CORES
    nc = _get_nc(S)
    in_maps = _stage_inputs(x, W, b)
    res = run_bass_kernel_spmd(nc, in_maps, list(range(N_CORES))).results
    return _unstage_outputs(res, S)


# revision 45
# speedup vs baseline: 1.0817x; 1.0032x over previous
"""NoisyTopkRouter Trainium2 kernel.

Computes, for x [8, 4096, 1024] f32, W [64, 1024], b [64]:
  logits = x @ W.T + b            [8, 4096, 64]
  top_vals, indices = top_k(logits, 8)
  probs = softmax(scatter(-inf, indices, top_vals))   (zeros off the top-8)
Returns (probs [8,4096,64] f32, indices [8,4096,8] int32).

Sharding: data-parallel over the batch dim — core i handles x[i] (4096
tokens).  Router weights are replicated.

All DRAM tensors are staged host-side as exact SBUF images so tiles move
with single large-descriptor DMAs:
  xs [NST*128, DC*ST]  xs[s*128+p, c*ST+t] = x[s*ST+t, 128c+p]
  ws [128, DC*E]       ws[p, c*E+e]        = W[e, 128c+p]
  bt [128, SUB*E]      bias replicated across partitions and subtiles
  p8_raw [NST*128, SUB*K], idx_raw [NST*128, SUB*K]  (device returns only
  the top-8 probs; host unstages by scattering them into the dense [T,64])

The softmax subtracts one shared per-row max (over the whole batched
tile) instead of a per-subtile max: any per-row constant cancels in the
exp(v)/sum(exp(top8)) ratio, and a single shift keeps the cross-engine
dependency chain short.
"""

import sys

try:  # the axon boot usually provides concourse already
    import concourse  # noqa: F401
except ImportError:
    sys.path.insert(0, "/opt/trn_rl_repo")

import numpy as np

D = 1024
E = 64
K = 8
P = 128
DC = D // P  # 8 d-chunks of 128
N_CORES = 8
ST = 512  # tokens per supertile (one DMA batch)
SUB = ST // P  # 128-token subtiles per supertile


def build_nc(T=4096):
    """Build + compile the per-core program for T tokens."""
    import concourse.bass as bass  # noqa: F401
    import concourse.tile as tile
    from concourse import bacc, mybir

    f32 = mybir.dt.float32
    u32 = mybir.dt.uint32
    Alu = mybir.AluOpType
    Act = mybir.ActivationFunctionType
    Axis = mybir.AxisListType

    assert T % ST == 0
    NST = T // ST

    nc = bacc.Bacc("TRN2", target_bir_lowering=False, debug=False)

    xs = nc.dram_tensor("xs", [NST * P, DC * ST], f32, kind="ExternalInput").ap()
    ws = nc.dram_tensor("ws", [P, DC * E], f32, kind="ExternalInput").ap()
    bt = nc.dram_tensor("bt", [P, (SUB // 2) * E], f32, kind="ExternalInput").ap()
    praw = nc.dram_tensor("praw", [NST * P, SUB * K], f32, kind="ExternalOutput").ap()
    iraw = nc.dram_tensor("iraw", [NST * P, SUB * K], u32, kind="ExternalOutput").ap()

    from contextlib import ExitStack

    with tile.TileContext(nc) as tc, ExitStack() as ctx:
        const = ctx.enter_context(tc.tile_pool(name="const", bufs=1))
        xpool = ctx.enter_context(tc.tile_pool(name="x", bufs=4))
        ppool = ctx.enter_context(tc.tile_pool(name="ps", bufs=4, space="PSUM"))
        opool = ctx.enter_context(tc.tile_pool(name="out", bufs=3))
        work = ctx.enter_context(tc.tile_pool(name="work", bufs=3))
        small = ctx.enter_context(tc.tile_pool(name="small", bufs=4))

        ws_sb = const.tile([P, DC * E], f32)
        bt_sb = const.tile([P, (SUB // 2) * E], f32)

        for s in range(NST):
            # supertile in 4 token-quarter DMAs so subtile u's matmuls can
            # start as soon as its 512KB lands (512B runs: full DMA rate)
            xt = xpool.tile([P, DC * ST], f32, tag="xst")
            xt3 = xt[:].rearrange("p (c t) -> p c t", t=ST)
            xs3 = xs[s * P : (s + 1) * P, :].rearrange("p (c t) -> p c t", t=ST)
            for u in range(SUB):
                if s == NST - 1 and u == SUB - 1:
                    # very last quarter split along d-chunks (8KB runs, full
                    # rate): the first 4 matmuls overlap the second half's
                    # transfer, starting the drain chain earlier
                    for ch in (slice(0, DC // 2), slice(DC // 2, DC)):
                        nc.sync.dma_start(
                            xt3[:, ch, u * P : (u + 1) * P],
                            xs3[:, ch, u * P : (u + 1) * P],
                        )
                else:
                    nc.sync.dma_start(
                        xt3[:, :, u * P : (u + 1) * P], xs3[:, :, u * P : (u + 1) * P]
                    )
            if s == 0:
                # const loads AFTER the first x quarters: the x stream owns
                # the DMA engines from t=0; ws still lands before the first
                # matmul group needs it
                nc.sync.dma_start(ws_sb[:], ws[:])
                nc.sync.dma_start(bt_sb[:], bt[:])
            po = opool.tile([P, SUB * K], f32, tag="po")
            io = opool.tile([P, SUB * K], u32, tag="io")
            # two PSUM tiles (= two banks) per supertile: PSUM hazards are
            # tracked bank-level, so one shared bank would serialize the
            # first half-tail behind the last subtile's matmuls
            HB = SUB // 2  # subtiles per half
            psA = ppool.tile([P, HB * E], f32, tag="psA")
            psB = ppool.tile([P, HB * E], f32, tag="psB")
            psh = [psA, psB]
            for u in range(SUB):
                for c in range(DC):
                    nc.tensor.matmul(
                        psh[u // HB][:, (u % HB) * E : (u % HB + 1) * E],
                        lhsT=xt[:, c * ST + u * P : c * ST + (u + 1) * P],
                        rhs=ws_sb[:, c * E : (c + 1) * E],
                        start=(c == 0),
                        stop=(c == DC - 1),
                    )
            # tail over subtile range [u0, u0+nsub): batched for steady-state
            # supertiles, per-subtile for the last one so the pipeline drains
            def tail(u0, nsub):
                lgn = work.tile([P, SUB * E], f32, tag="lg")
                lg = lgn[:, : nsub * E]
                # bias-add per PSUM half so each half-tail gates only on its
                # own bank's matmuls
                for h0 in range(u0, u0 + nsub, HB):
                    n = min(HB, u0 + nsub - h0)
                    nc.vector.tensor_add(
                        lg[:, (h0 - u0) * E : (h0 - u0 + n) * E],
                        psh[h0 // HB][:, (h0 % HB) * E : (h0 % HB + n) * E],
                        bt_sb[:, : n * E],
                    )
                v8n = small.tile([P, SUB * K], f32, tag="v8")
                for j in range(nsub):
                    nc.vector.max(
                        out=v8n[:, j * K : (j + 1) * K],
                        in_=lg[:, j * E : (j + 1) * E],
                    )
                # shared per-row shift keeps every exp argument in [-x, 0];
                # any per-row constant cancels in the softmax ratio
                nm = small.tile([P, 1], f32, tag="nm")
                nc.vector.tensor_reduce(
                    nm[:], lg, axis=Axis.X, op=Alu.max, negate=True
                )
                for j in range(nsub):
                    nc.vector.max_index(
                        io[:, (u0 + j) * K : (u0 + j + 1) * K],
                        v8n[:, j * K : (j + 1) * K],
                        lg[:, j * E : (j + 1) * E],
                    )
                # denominators: sum over each subtile's top-8 exps
                e8n = small.tile([P, SUB * K], f32, tag="e8")
                nc.scalar.activation(
                    e8n[:, : nsub * K], v8n[:, : nsub * K], Act.Exp, bias=nm[:]
                )
                s8n = small.tile([P, SUB], f32, tag="s8")
                nc.vector.tensor_reduce(
                    s8n[:, :nsub],
                    e8n[:, : nsub * K].rearrange("p (u k) -> p u k", k=K),
                    axis=Axis.X,
                    op=Alu.add,
                )
                rcn = small.tile([P, SUB], f32, tag="rc")
                nc.vector.reciprocal(rcn[:, :nsub], s8n[:, :nsub])
                # compact output: only the top-8 probs, p8 = exp(v8-m)/sum8;
                # the dense [T,64] tensor is scattered on the host
                for j in range(nsub):
                    nc.vector.tensor_scalar(
                        out=po[:, (u0 + j) * K : (u0 + j + 1) * K],
                        in0=e8n[:, j * K : (j + 1) * K],
                        scalar1=rcn[:, j : j + 1],
                        scalar2=None,
                        op0=Alu.mult,
                    )

            if s < NST - 1:
                tail(0, SUB)
                # SWDGE (gpsimd) queues: keeps stores off the SP sequencer so
                # the next supertile's x load is never head-of-line blocked.
                nc.gpsimd.dma_start(praw[s * P : (s + 1) * P, :], po[:])
                nc.gpsimd.dma_start(iraw[s * P : (s + 1) * P, :], io[:])
            else:
                # drain the pipeline in halves; SP + HWDGE are idle by now
                h = SUB // 2
                tail(0, h)
                nc.sync.dma_start(praw[s * P : (s + 1) * P, : h * K], po[:, : h * K])
                nc.sync.dma_start(iraw[s * P : (s + 1) * P, : h * K], io[:, : h * K])
                tail(h, SUB - h)
                nc.sync.dma_start(praw[s * P : (s + 1) * P, h * K :], po[:, h * K :])
                nc.sync.dma_start(iraw[s * P : (s + 1) * P, h * K :], io[:, h * K :])

    nc.compile()
    return nc


_NC_CACHE = {}


def _get_nc(T=4096):
    if T not in _NC_CACHE:
        _NC_CACHE[T] = build_nc(T)
    return _NC_CACHE[T]


def _stage_inputs(x, W, b):
    x = np.asarray(x, dtype=np.float32)
    W = np.asarray(W, dtype=np.float32)
    b = np.asarray(b, dtype=np.float32)
    B, S, d = x.shape
    assert d == D and W.shape == (E, D) and b.shape == (E,)
    NST = S // ST
    # ws[p, c*E+e] = W[e, 128c+p]
    ws = np.ascontiguousarray(W.T.reshape(DC, P, E).transpose(1, 0, 2).reshape(P, DC * E))
    bt = np.ascontiguousarray(np.broadcast_to(np.tile(b, SUB // 2)[None, :], (P, (SUB // 2) * E)))
    in_maps = []
    for i in range(N_CORES):
        # xs[s*128+p, c*ST+t] = x[i, s*ST+t, 128c+p]
        xi = x[i].reshape(NST, ST, DC, P)  # [s, t, c, p]
        xsi = np.ascontiguousarray(xi.transpose(0, 3, 2, 1).reshape(NST * P, DC * ST))
        in_maps.append({"xs": xsi, "ws": ws, "bt": bt})
    return in_maps


def _unstage_outputs(res, S):
    NST = S // ST
    probs = np.empty((N_CORES, S, E), dtype=np.float32)
    indices = np.empty((N_CORES, S, K), dtype=np.int32)
    for i in range(N_CORES):
        p8 = res[i]["praw"].reshape(NST, P, SUB, K).transpose(0, 2, 1, 3).reshape(S, K)
        ir = res[i]["iraw"].view(np.int32).reshape(NST, P, SUB, K).transpose(0, 2, 1, 3)
        indices[i] = ir.reshape(S, K)
        probs[i] = 0.0
        np.put_along_axis(probs[i], indices[i], p8, axis=1)
    return probs, indices


def kernel(x, W, b):
    from concourse.bass_utils import run_bass_kernel_spmd

    x = np.asarray(x, dtype=np.float32)
    B, S, d = x.shape
    assert B == N_CORES
    nc = _get_nc(S)
    in_maps = _stage_inputs(x, W, b)
    res = run_bass_kernel_spmd(nc, in_maps, list(range(N_CORES))).results
    return _unstage_outputs(res, S)
